# revision 3
# baseline (speedup 1.0000x reference)
"""DiceLoss kernel v2 for 8 Trainium2 NeuronCores — fp8 + PE-centric.

Reference computation:
    inter[b,c] = sum_p pred[b,c,p] * target[b,c,p]          # [4, 8]
    denom      = sum(pred) + sum(target) + 1.0              # scalar
    loss_bc    = 2 * (inter + 1) / denom
    total      = sum_b( sum_c(loss_bc[b]) * 8**(b-4) ) / 4
    out        = 1 - total

Numerics: inputs are uniform[0,1); host-casting them to fp8-e4m3 gives
rel err ~2e-8 on the final loss (quantization noise averages out over
16.8M samples; the 1-total leverage shrinks it further) — far inside
the 2e-2 gate while quartering HBM traffic, the binding constraint.

Sharding: flatten (b,c) -> 32 rows of 2M pixels; core k takes rows
4k..4k+3 ("groups" 0..3).  Per group the 2M pixels are packed into 130
PE tiles of [K=128 partitions, 128 cols] where col 127 of every tile
is 1.0 (and the data tail is zero-padded).  For each tile the PE runs
matmul(psum_g, lhsT=pred_tile, rhs=targ_tile) accumulating all 130
tiles of a group into one [128,128] PSUM region:
    psum_g[i,j]   = sum_t sum_k pred[k,t,i] * targ[k,t,j]
    diag(psum_g)  -> per-column dot products   (sum -> inter[g])
    psum_g[:,127] -> per-column pred sums      (sum -> sum(pred))
    psum_g[127,:] -> per-column targ sums      (sum -> sum(targ))
so the PE computes the dots AND both global sums in one fp8 matmul
stream (~35us/core), under the fp8 DMA floor (~47us/core).  DVE only
extracts: diag via a masked reduce against an identity matrix, plus a
column copy and a row reduce per group (<1us total).  Host folds the
[128,12] per-core result into the scalar loss.
"""

from contextlib import ExitStack

import numpy as np
import ml_dtypes

N, C, P = 4, 8, 2097152
NCORES = 8
ROWS = N * C                      # 32 (b,c) rows
RPC = ROWS // NCORES              # 4 rows (groups) per core
COLS = P // 128                   # 16384 data cols per group
NT = 130                          # PE tiles per group (130*127 >= 16384)
TPG = NT * 128                    # 16640 slab cols per group
SLAB_W = RPC * TPG                # 66560 slab cols per core
GUARD = 16                        # unread guard rows flanking dram slabs
SUBTS = [52, 52, 26]              # tiles per DMA sub-piece (sum = NT)
SUB = len(SUBTS)
SUBO = [sum(SUBTS[:i]) for i in range(SUB)]   # tile offsets

F8 = ml_dtypes.float8_e4m3

_CACHE = {}


def _build_bass():
    import concourse.bass as bass
    import concourse.mybir as mybir

    f32 = mybir.dt.float32
    f8 = mybir.dt.float8e4
    nc = bass.Bass("TRN2", target_bir_lowering=False, debug=False,
                   num_devices=NCORES)

    pred = nc.dram_tensor("pred", [128 + 2 * GUARD, SLAB_W], f8,
                          kind="ExternalInput").ap()
    targ = nc.dram_tensor("target", [128 + 2 * GUARD, SLAB_W], f8,
                          kind="ExternalInput").ap()
    ident = nc.dram_tensor("ident", [128, 128], f32,
                           kind="ExternalInput").ap()
    out = nc.dram_tensor("out", [128, 12], f32, kind="ExternalOutput").ap()

    predf = pred[GUARD:GUARD + 128, :]
    targf = targ[GUARD:GUARD + 128, :]

    AX = mybir.AxisListType.X
    MUL = mybir.AluOpType.mult

    with ExitStack() as ctx:
        e = ctx.enter_context
        pred_sl = [e(nc.sbuf_tensor(f"pred_sl{i}", [128, TPG], f8))
                   for i in range(2)]
        targ_sl = [e(nc.sbuf_tensor(f"targ_sl{i}", [128, TPG], f8))
                   for i in range(2)]
        ident_sb = e(nc.sbuf_tensor([128, 128], f32))
        finals = e(nc.sbuf_tensor([128, 12], f32))
        dummy = e(nc.sbuf_tensor([128, 1], f32))
        ps = [e(nc.psum_tensor(f"ps{g}", [128, 128], f32))
              for g in range(RPC)]

        # one sem per (slot, sub-piece): every wait threshold equals the
        # total inc count of ALL DMAs ever issued on that sem, so a lagging
        # SDMA engine cannot be masked by faster engines racing ahead
        # (15 engines x all chunks < 16 x chunks_needed).
        ss = [[e(nc.semaphore(f"ss{p}_{s}")) for s in range(SUB)]
              for p in range(2)]
        s_id = e(nc.semaphore())   # ident loaded
        s_pe = e(nc.semaphore())   # PE groups done
        s_dve = e(nc.semaphore())  # DVE groups extracted
        s_out = e(nc.semaphore())  # output stored

        block = e(nc.Block(no_gpsimd_drain=True))

        # pred pieces ride the Sync HWDGE ring, targ pieces the Scalar
        # (ACT) HWDGE ring: two issue queues feeding the 16 SDMA engines,
        # halving the per-ring DIRECT2D issue load so the engines never
        # starve on descriptor generation.
        @block.sync
        def _(sync):
            for g in range(RPC):
                p = g % 2
                if g >= 2:
                    sync.wait_ge(s_pe, g - 1)   # slot's previous group done
                base = g * TPG
                for s in range(SUB):
                    o = SUBO[s] * 128
                    w = SUBTS[s] * 128
                    sync.dma_start(
                        pred_sl[p][:, o:o + w],
                        predf[:, base + o:base + o + w],
                    ).then_inc(ss[p][s], 16)
                if g == 0:
                    sync.dma_start(ident_sb[:], ident).then_inc(s_id, 16)
            sync.wait_ge(s_dve, RPC)
            sync.dma_start(out, finals[:]).then_inc(s_out, 16)

        @block.tensor
        def _(tensor):
            for g in range(RPC):
                p = g % 2
                gen = g // 2
                for s in range(SUB):
                    tensor.wait_ge(ss[p][s], 32 * (gen + 1))
                    for t in range(SUBTS[s]):
                        ti = SUBO[s] + t
                        mm = nc.tensor.matmul(
                            ps[g][:],
                            pred_sl[p][:, ti * 128:(ti + 1) * 128],
                            targ_sl[p][:, ti * 128:(ti + 1) * 128],
                            start=(ti == 0),
                            stop=(ti == NT - 1),
                        )
                        if ti == NT - 1:
                            mm.then_inc(s_pe, 1)

        @block.vector
        def _(vector):
            nc.vector.memset(finals[:], 0.0)
            vector.wait_ge(s_id, 16)
            for g in range(RPC):
                vector.wait_ge(s_pe, g + 1)
                nc.vector.scalar_tensor_tensor(
                    out=dummy[:, 0:1].broadcast_to((128, 128)),
                    in0=ps[g][:],
                    scalar=1.0,
                    in1=ident_sb[:],
                    op0=MUL,
                    op1=MUL,
                    accum_out=finals[:, g:g + 1],
                )
                nc.vector.tensor_copy(finals[:, 4 + g:5 + g],
                                      ps[g][:, 127:128])
                nc.vector.reduce_sum(finals[:, 8 + g:9 + g],
                                     ps[g][:, 0:127],
                                     axis=AX).then_inc(s_dve, 1)

        @block.scalar
        def _(scalar):
            for g in range(RPC):
                p = g % 2
                if g >= 2:
                    scalar.wait_ge(s_pe, g - 1)
                base = g * TPG
                for s in range(SUB):
                    o = SUBO[s] * 128
                    w = SUBTS[s] * 128
                    scalar.dma_start(
                        targ_sl[p][:, o:o + w],
                        targf[:, base + o:base + o + w],
                    ).then_inc(ss[p][s], 16)

    return nc


def _pack(core_rows: np.ndarray) -> np.ndarray:
    """[RPC, P] fp8 rows -> guarded [128+2G, SLAB_W] fp8 slab."""
    slab = np.zeros((128 + 2 * GUARD, SLAB_W), dtype=F8)
    body = slab[GUARD:GUARD + 128]
    one = np.array(1.0, dtype=F8)
    packed = np.zeros((128, NT, 128), dtype=F8)
    pad = np.zeros((128, NT * 127), dtype=F8)
    for g in range(RPC):
        pad[:, :COLS] = core_rows[g].reshape(COLS, 128).T
        packed[:, :, :127] = pad.reshape(128, NT, 127)
        packed[:, :, 127] = one
        body[:, g * TPG:(g + 1) * TPG] = packed.reshape(128, TPG)
    return slab


def _make_in_maps(pred: np.ndarray, target: np.ndarray):
    predr = np.asarray(pred, dtype=np.float32).reshape(ROWS, P).astype(F8)
    targr = np.asarray(target, dtype=np.float32).reshape(ROWS, P).astype(F8)
    ident = np.eye(128, dtype=np.float32)
    maps = []
    for k in range(NCORES):
        maps.append({
            "pred": _pack(predr[k * RPC:(k + 1) * RPC]),
            "target": _pack(targr[k * RPC:(k + 1) * RPC]),
            "ident": ident,
        })
    return maps


def _run(pred: np.ndarray, target: np.ndarray, trace: bool = False):
    from concourse.bass_utils import run_bass_kernel_spmd

    if "nc" not in _CACHE:
        _CACHE["nc"] = _build_bass()
    nc = _CACHE["nc"]
    in_maps = _make_in_maps(pred, target)
    return run_bass_kernel_spmd(nc, in_maps, core_ids=list(range(NCORES)),
                                trace=trace)


def _combine(results) -> np.ndarray:
    inter = np.empty(ROWS, dtype=np.float64)
    sums = 0.0
    for k in range(NCORES):
        o = np.asarray(results[k]["out"], dtype=np.float64)   # [128, 12]
        for g in range(RPC):
            inter[k * RPC + g] = o[0:127, g].sum()
            sums += o[0:127, 4 + g].sum() + o[127, 8 + g]
    denom = sums + 1.0
    loss_bc = 2.0 * (inter.reshape(N, C) + 1.0) / denom
    weights = np.float64(C) ** (np.arange(N, dtype=np.float64) - N)
    total = (loss_bc.sum(axis=1) * weights).sum() / N
    return np.array(1.0 - total, dtype=np.float32)


def kernel(pred: np.ndarray, target: np.ndarray) -> np.ndarray:
    res = _run(pred, target, trace=False)
    return _combine(res.results)


# revision 6
# speedup vs baseline: 1.0047x; 1.0047x over previous
"""DiceLoss kernel v10 for 8 Trainium2 NeuronCores — fp8, PE + DVE
split, fully-resident SBUF (no flow control) + short final PE burst.

Reference computation:
    inter[b,c] = sum_p pred[b,c,p] * target[b,c,p]          # [4, 8]
    denom      = sum(pred) + sum(target) + 1.0              # scalar
    loss_bc    = 2 * (inter + 1) / denom
    total      = sum_b( sum_c(loss_bc[b]) * 8**(b-4) ) / 4
    out        = 1 - total

Numerics: inputs are uniform[0,1); host-casting to fp8-e4m3 gives rel
err ~6e-8 on the final loss (quantization noise averages out over
16.8M samples) — far inside the 2e-2 gate while quartering HBM
traffic, the binding constraint for this memory-bound problem.

Sharding: flatten (b,c) -> 32 rows of 2M pixels; core k takes rows
4k..4k+3 ("groups" 0..3).  Each group's pixels split into:

  A region (86%): 112 PE tiles of [K=128, 128 cols], col 127 = 1.0.
    matmul(psum_g, lhsT=pred_tile, rhs=targ_tile) accumulates all A
    tiles of a group into one [128,128] PSUM region:
      diag(psum_g)  -> per-column dots,  psum_g[:,127] -> pred sums,
      psum_g[127,:] -> targ sums
    — dots AND both global sums in one fp8 matmul stream (~35us/core).

  B region (14%): a raw [112, 2469] slab on partitions [8:120].  The
    otherwise-idle DVE computes its dot (scalar_tensor_tensor
    mult/mult with per-partition accum) and combined pred+targ sum
    (add/add) — no tile structure, no ones columns (~21us, hidden).

DMA shaping: HWDGE splits a [0:128] piece over 16 SDMA engines by
partition octet (engine k <- partitions 8k..8k+7).  Traces show engine
0 or engine 15 of a core intermittently running at ~0.8x, straggling
past the others and stalling the stream tail.  The B region lives on
partitions [8:120] -> engines 1-14 only, putting ~0.85x load on both
edge engines; per group the B piece is issued FIRST (so DVE overlaps
PE) and the last A piece is small (22 tiles) to keep the final PE
burst short.  Issue load is split across both HWDGE rings (pred on
Sync, targ on Scalar) so descriptor generation never starves the
engines.  Per-(slot, piece) semaphores make every wait threshold equal
to the total inc count of ALL DMAs ever issued on that sem, so a
lagging SDMA engine cannot be masked by faster engines racing ahead.

Host folds the [128, 20] per-core results into the scalar loss.
"""

from contextlib import ExitStack

import numpy as np
import ml_dtypes

N, C, P = 4, 8, 2097152
NCORES = 8
ROWS = N * C                      # 32 (b,c) rows
RPC = ROWS // NCORES              # 4 rows (groups) per core
NA = 112                          # A-tiles per group (K=128)
APX = NA * 127 * 128              # pixels in A region (1820672, exact cols)
BPX = P - APX                     # pixels in B region (276480)
BK0, BK1 = 0, 128                 # B-region partition window (all engines)
BK = BK1 - BK0                    # 112
BW = -(-BPX // BK)                # 2469 B-region cols
AW = NA * 128                     # A region cols per group (14336)
TPG = AW + BW                     # slab cols per group (16805)
SLAB_W = RPC * TPG                # 67220 slab cols per core
GUARD = 16                        # unread guard rows flanking dram slabs
SUBTS = [49, 49, 10, 4]           # A-piece tile counts per group
                                  # (tiny last piece: after the straggling
                                  # SDMA engine delivers its final chunk,
                                  # only a 4-tile PE burst remains)

F8 = ml_dtypes.float8_e4m3

_CACHE = {}


def _build_bass():
    import concourse.bass as bass
    import concourse.mybir as mybir

    f32 = mybir.dt.float32
    f8 = mybir.dt.float8e4
    nc = bass.Bass("TRN2", target_bir_lowering=False, debug=False,
                   num_devices=NCORES)

    pred = nc.dram_tensor("pred", [128 + 2 * GUARD, SLAB_W], f8,
                          kind="ExternalInput").ap()
    targ = nc.dram_tensor("target", [128 + 2 * GUARD, SLAB_W], f8,
                          kind="ExternalInput").ap()
    ident = nc.dram_tensor("ident", [128, 128], f32,
                           kind="ExternalInput").ap()
    out = nc.dram_tensor("out", [128, 20], f32, kind="ExternalOutput").ap()

    predf = pred[GUARD:GUARD + 128, :]
    targf = targ[GUARD:GUARD + 128, :]

    AX = mybir.AxisListType.X
    MUL = mybir.AluOpType.mult
    ADD = mybir.AluOpType.add

    with ExitStack() as ctx:
        e = ctx.enter_context
        # all 4 groups resident at once (8 x 16.5KB/partition = 132KB of
        # SBUF): every DMA is issued up front with no flow control, so the
        # SDMA engines never starve on sequencer waits mid-stream.
        pred_sl = [e(nc.sbuf_tensor(f"pred_sl{i}", [128, TPG], f8))
                   for i in range(RPC)]
        targ_sl = [e(nc.sbuf_tensor(f"targ_sl{i}", [128, TPG], f8))
                   for i in range(RPC)]
        ident_sb = e(nc.sbuf_tensor([128, 128], f32))
        finals = e(nc.sbuf_tensor([128, 20], f32))
        dummy = e(nc.sbuf_tensor([128, 2], f32))
        ps = [e(nc.psum_tensor(f"ps{g}", [128, 128], f32))
              for g in range(RPC)]

        # one sem per (group, piece): each sem sees exactly one pred+targ
        # DMA pair ever, so waiting for 32 equals all-DMAs-ever-issued and
        # a lagging SDMA engine cannot be masked by faster engines.
        ss = [[e(nc.semaphore(f"ss{g}_{s}")) for s in range(1 + len(SUBTS))]
              for g in range(RPC)]
        s_id = e(nc.semaphore())   # ident loaded
        s_pe = e(nc.semaphore())   # PE groups done
        s_dve = e(nc.semaphore())  # DVE groups done (B STTs + extraction)
        s_out = e(nc.semaphore())  # output stored

        block = e(nc.Block(no_gpsimd_drain=True))

        def issue_pieces(eng, src_ap, slots, g):
            base = g * TPG
            eng.dma_start(
                slots[g][BK0:BK1, AW:AW + BW],
                src_ap[BK0:BK1, base + AW:base + AW + BW],
            ).then_inc(ss[g][0], 16)
            o = 0
            for s, nt in enumerate(SUBTS):
                w = nt * 128
                eng.dma_start(
                    slots[g][:, o:o + w],
                    src_ap[:, base + o:base + o + w],
                ).then_inc(ss[g][1 + s], 16)
                o += w

        @block.sync
        def _(sync):
            for g in range(RPC):
                issue_pieces(sync, predf, pred_sl, g)
                if g == 0:
                    sync.dma_start(ident_sb[:], ident).then_inc(s_id, 16)
            sync.wait_ge(s_dve, RPC)
            sync.dma_start(out, finals[:]).then_inc(s_out, 16)

        @block.scalar
        def _(scalar):
            for g in range(RPC):
                issue_pieces(scalar, targf, targ_sl, g)

        @block.tensor
        def _(tensor):
            for g in range(RPC):
                ti = 0
                for s, nt in enumerate(SUBTS):
                    tensor.wait_ge(ss[g][1 + s], 32)
                    for _ in range(nt):
                        mm = nc.tensor.matmul(
                            ps[g][:],
                            pred_sl[g][:, ti * 128:(ti + 1) * 128],
                            targ_sl[g][:, ti * 128:(ti + 1) * 128],
                            start=(ti == 0),
                            stop=(ti == NA - 1),
                        )
                        if ti == NA - 1:
                            mm.then_inc(s_pe, 1)
                        ti += 1

        @block.vector
        def _(vector):
            nc.vector.memset(finals[:], 0.0)
            vector.wait_ge(s_id, 16)
            for g in range(RPC):
                # B region: dot + combined sum as soon as its piece lands
                vector.wait_ge(ss[g][0], 32)
                nc.vector.scalar_tensor_tensor(
                    out=dummy[BK0:BK1, 0:1].broadcast_to((BK, BW)),
                    in0=pred_sl[g][BK0:BK1, AW:AW + BW],
                    scalar=1.0,
                    in1=targ_sl[g][BK0:BK1, AW:AW + BW],
                    op0=MUL,
                    op1=MUL,
                    accum_out=finals[BK0:BK1, 12 + g:13 + g],
                )
                nc.vector.scalar_tensor_tensor(
                    out=dummy[BK0:BK1, 1:2].broadcast_to((BK, BW)),
                    in0=pred_sl[g][BK0:BK1, AW:AW + BW],
                    scalar=0.0,
                    in1=targ_sl[g][BK0:BK1, AW:AW + BW],
                    op0=ADD,
                    op1=ADD,
                    accum_out=finals[BK0:BK1, 16 + g:17 + g],
                )
                # A region: extract psum once the PE finishes the group
                vector.wait_ge(s_pe, g + 1)
                nc.vector.scalar_tensor_tensor(
                    out=dummy[:, 0:1].broadcast_to((128, 128)),
                    in0=ps[g][:],
                    scalar=1.0,
                    in1=ident_sb[:],
                    op0=MUL,
                    op1=MUL,
                    accum_out=finals[:, g:g + 1],
                )
                nc.vector.tensor_copy(finals[:, 4 + g:5 + g],
                                      ps[g][:, 127:128])
                nc.vector.reduce_sum(finals[:, 8 + g:9 + g],
                                     ps[g][:, 0:127],
                                     axis=AX).then_inc(s_dve, 1)

    return nc


def _pack(core_rows: np.ndarray) -> np.ndarray:
    """[RPC, P] fp8 rows -> guarded [128+2G, SLAB_W] fp8 slab."""
    slab = np.zeros((128 + 2 * GUARD, SLAB_W), dtype=F8)
    body = slab[GUARD:GUARD + 128]
    one = np.array(1.0, dtype=F8)
    apacked = np.zeros((128, NA, 128), dtype=F8)
    for g in range(RPC):
        d = core_rows[g]
        # A region: first APX pixels; exactly NA*127 data cols
        apacked[:, :, :127] = (
            d[:APX].reshape(NA * 127, 128).T.reshape(128, NA, 127)
        )
        apacked[:, :, 127] = one
        body[:, g * TPG:g * TPG + AW] = apacked.reshape(128, AW)
        # B region: remaining pixels, raw [BK, BW] on partitions [8:120]
        btmp = np.zeros(BK * BW, dtype=F8)
        btmp[:BPX] = d[APX:]
        body[BK0:BK1, g * TPG + AW:(g + 1) * TPG] = (
            btmp.reshape(BW, BK).T
        )
    return slab


def _make_in_maps(pred: np.ndarray, target: np.ndarray):
    predr = np.asarray(pred, dtype=np.float32).reshape(ROWS, P).astype(F8)
    targr = np.asarray(target, dtype=np.float32).reshape(ROWS, P).astype(F8)
    ident = np.eye(128, dtype=np.float32)
    maps = []
    for k in range(NCORES):
        maps.append({
            "pred": _pack(predr[k * RPC:(k + 1) * RPC]),
            "target": _pack(targr[k * RPC:(k + 1) * RPC]),
            "ident": ident,
        })
    return maps


def _run(pred: np.ndarray, target: np.ndarray, trace: bool = False):
    from concourse.bass_utils import run_bass_kernel_spmd

    if "nc" not in _CACHE:
        _CACHE["nc"] = _build_bass()
    nc = _CACHE["nc"]
    in_maps = _make_in_maps(pred, target)
    return run_bass_kernel_spmd(nc, in_maps, core_ids=list(range(NCORES)),
                                trace=trace)


def _combine(results) -> np.ndarray:
    inter = np.empty(ROWS, dtype=np.float64)
    sums = 0.0
    for k in range(NCORES):
        o = np.asarray(results[k]["out"], dtype=np.float64)   # [128, 20]
        for g in range(RPC):
            inter[k * RPC + g] = o[0:127, g].sum() + o[:, 12 + g].sum()
            sums += (o[0:127, 4 + g].sum() + o[127, 8 + g]
                     + o[:, 16 + g].sum())
    denom = sums + 1.0
    loss_bc = 2.0 * (inter.reshape(N, C) + 1.0) / denom
    weights = np.float64(C) ** (np.arange(N, dtype=np.float64) - N)
    total = (loss_bc.sum(axis=1) * weights).sum() / N
    return np.array(1.0 - total, dtype=np.float32)


def kernel(pred: np.ndarray, target: np.ndarray) -> np.ndarray:
    res = _run(pred, target, trace=False)
    return _combine(res.results)
